# revision 1
# baseline (speedup 1.0000x reference)
"""Self-contained Trainium2 Bass kernel for nn_GCNResnet (batch-attention GCN).

Math (reference collapse):
  out[b,:] = sum_n c_n * softmax(X_n X_n^T)[b,:] @ (X_n @ W) + offset*(1_D @ W)
where X_n = x[:, n, :] ([B=4096, D=10]); c_n and offset fold BN(eval) +
adjacency + GCN + avgpool scalars. Per node the softmax normalizer is folded
into the PV matmul via a ones column:
  U_n = exp(X_n X_n^T) @ [c_n*(X_n@W) | 1]   -> out_n = U[:, :10] / U[:, 10]
(no max-subtraction needed: |scores| <= ~45 << 88, exp stays in fp32 range).

Sharding: row-slab parallel over 8 cores, 512 query rows per core, no
collectives. Each core computes S^T chunks [128 keys x 512 rows] on the PE
(fp32r operands -> full rate), exp on the scalar engine (the throughput wall:
128 lanes @ 1.2 GHz), and accumulates U^T [11 x 512] per node in PSUM.
The tiny final divide+transpose (0.08% of FLOPs) happens on host during the
gather/unshard step.

PSUM layout: two 3-bank scores buffers double-buffering each other (PE fills
one while ACT exps the other) + 1 bank for the U accumulator. Scores matmuls
are PE row-tiled (tile_position=(32i,0), K=10 each) so a group's 3 matmuls
run concurrently on disjoint 32-row strips of the array.
"""

import sys

if "/opt/trn_rl_repo" not in sys.path:
    sys.path.insert(0, "/opt/trn_rl_repo")

import numpy as np

import concourse.bass as bass
import concourse.mybir as mybir
from concourse import tile
from concourse.bass_utils import run_bass_kernel_spmd
from concourse.vector_clock import ScopedClock

B, N, D = 4096, 3, 10
NCORES = 8
R = B // NCORES            # 512 query rows per core
KC = B // 128              # 32 key chunks of 128
BN_EPS = 1e-5

# perf/numerics switches (exercised by test.py)
USE_F32R_SCORES = True
USE_F32R_PV = True
ROW_TILE = True
XT_BUFS = 2
U_BUFS = 1
ET_BUFS = 3
EXP_SPLIT = 1              # activation calls per scores group (diagnostic)
GROUP_PATTERN = (3, 3)     # scores-group widths; two 3-bank psum buffers double-buffer


def _patched_drain_and_barrier(self, tick_clock, wait_clock):
    # Walrus in this container rejects >1 sync-wait on a CTRL-class
    # instruction; absorb the tail-drain waits into SP nops, one wait each.
    nc = self.nc
    probe = nc.sync.nop()
    wait_clock.add_sem_waits(probe.ins, ScopedClock({None: tick_clock.global_clock}))
    si = probe.ins.sync_info
    waits = list(si.on_wait) if si is not None else []
    upds = list(si.on_update) if si is not None else []
    probe.ins.sync_info = mybir.SyncInfo(on_wait=waits[:1], on_update=upds)
    for w in waits[1:]:
        n = nc.sync.nop()
        n.ins.sync_info = mybir.SyncInfo(on_wait=[w], on_update=[])
    nc.sync.drain()
    nc.all_engine_barrier()
    assert self.sems is not None
    popped = nc._tile_sem_poison_stack.pop()
    assert popped is self._sem_poison
    nc.clear_and_free_semaphores(list(self.sems.allocated().values()))
    nc.all_engine_barrier()


tile.TileContext._drain_and_barrier = _patched_drain_and_barrier

_MAX_WAITS = 1
_waitsplit_ctr = [0]


def _split_sync_waits(nc):
    """Walrus here allows very few sync-waits per instruction. Move excess
    waits onto same-engine no-ops placed immediately before the instruction
    (engine streams are in-order, so semantics are preserved)."""
    for f in nc.m.functions:
        for bb in f.blocks:
            new = []
            changed = False
            for inst in bb.instructions:
                si = inst.sync_info
                waits = list(si.on_wait) if si is not None else []
                if len(waits) > _MAX_WAITS:
                    changed = True
                    for w in waits[:-_MAX_WAITS]:
                        _waitsplit_ctr[0] += 1
                        nop = mybir.InstNoOp(
                            name=f"I-waitsplit-{_waitsplit_ctr[0]}", ins=[], outs=[]
                        )
                        nop.engine = inst.engine
                        nop.sync_info = mybir.SyncInfo(on_wait=[w], on_update=[])
                        new.append(nop)
                    inst.sync_info = mybir.SyncInfo(
                        on_wait=waits[-_MAX_WAITS:], on_update=list(si.on_update)
                    )
                new.append(inst)
            if changed:
                bb.instructions = new


def _mdt(use_f32r):
    # float32r is reduced-mantissa fp32 (TF32-like): matmul runs at full rate
    # (1 cycle/row vs 4 for fp32, moving dim >= 256) at ~1.8e-4 operand
    # rounding. Producer instructions must write fp32r (gpsimd casting DMA /
    # ACT output dtype); bitcasting raw fp32 is rejected by the BIR verifier.
    return mybir.dt.float32r if use_f32r else mybir.dt.float32


def _groups():
    """Key-chunk groups with alternating widths and their psum pool parity."""
    gs, c, gi = [], 0, 0
    while c < KC:
        w = GROUP_PATTERN[gi % len(GROUP_PATTERN)]
        w = min(w, KC - c)
        gs.append((list(range(c, c + w)), gi % 2))
        c += w
        gi += 1
    return gs


def build_nc(rep: int = 1, rep_marker: bool = False, mode: str = "full") -> bass.Bass:
    """One-core SPMD program: full keys replicated, this core's 512-row slab.

    mode: "full" (loads+compute per rep), "loads" (DMAs only per rep),
    "compute" (loads once, compute per rep) — for timing decomposition.
    """
    f32 = mybir.dt.float32
    nc = bass.Bass()

    # per-core input: all keys' X^T plus this core's query slab, concatenated
    # along the column axis so each (node, replica) loads with one DMA
    xtc = nc.declare_dram_parameter("xtc", [N, D, B + R], f32, isOutput=False)
    xh = nc.declare_dram_parameter("xh", [N, KC, 128, D + 1], f32, isOutput=False)
    uout = nc.declare_dram_parameter("uout", [D + 2, 512 * N], f32, isOutput=True)

    # partition replicas of xt for PE row tiling (concurrent row tiles)
    n_rt = min(max(GROUP_PATTERN), 4) if ROW_TILE else 1
    groups = _groups()

    with tile.TileContext(nc) as tc:
        with (
            tc.tile_pool(name="xtp", bufs=XT_BUFS) as xtp,
            tc.tile_pool(name="xhp", bufs=XT_BUFS) as xhp,
            tc.tile_pool(name="etp", bufs=ET_BUFS) as etp,
            tc.tile_pool(name="mrk", bufs=1) as mrkp,
            tc.tile_pool(name="pssA", bufs=1, space="PSUM") as pssA,
            tc.tile_pool(name="pssB", bufs=1, space="PSUM") as pssB,
            tc.tile_pool(name="psu", bufs=U_BUFS, space="PSUM") as psu,
        ):
            sdt = _mdt(USE_F32R_SCORES)
            pdt = _mdt(USE_F32R_PV)
            xt_sb = xh_sb = None
            for rep_i in range(rep):
                if mode != "compute" or rep_i == 0:
                    # ---- input loads ----
                    # columns 0..B-1: all keys; columns B..B+R-1: this core's
                    # query slab (replicated per row-tile partition offset).
                    # gpsimd DMAs cast fp32 -> fp32r on the fly.
                    xt_sb = [
                        xtp.tile([128, B + R], sdt, tag=f"xt{n}", name=f"xt{n}")
                        for n in range(N)
                    ]
                    xt_eng = nc.gpsimd if USE_F32R_SCORES else nc.sync
                    for n in range(N):
                        for i in range(n_rt):
                            xt_eng.dma_start(xt_sb[n][32 * i : 32 * i + D, :], xtc[n])
                    xh_sb = xhp.tile([128, N * KC * (D + 1)], pdt)
                    xh_r = xh_sb[:].rearrange("p (n c d) -> p n c d", n=N, c=KC)
                    xh_eng = nc.gpsimd if USE_F32R_PV else nc.sync
                    for n in range(N):
                        xh_eng.dma_start(xh_r[:, n], xh[n].rearrange("c p d -> p c d"))
                if mode == "loads":
                    continue

                for n in range(N):
                    u_ps = psu.tile([128, 512], f32, tag="u")
                    for g, parity in groups:
                        w = len(g)
                        pool = pssB if parity else pssA
                        wmax = GROUP_PATTERN[parity % len(GROUP_PATTERN)]
                        ps = pool.tile(
                            [128, 512 * wmax], f32, tag=f"s{parity}", name=f"s{parity}"
                        )
                        for i, ck in enumerate(g):
                            po = 32 * i if ROW_TILE else 0
                            nc.tensor.matmul(
                                ps[:, 512 * i : 512 * (i + 1)],
                                lhsT=xt_sb[n][po : po + D, 128 * ck : 128 * (ck + 1)],
                                rhs=xt_sb[n][po : po + D, B : B + R],
                                tile_position=(po, 0),
                            )
                        et = etp.tile([128, 512 * max(GROUP_PATTERN)], pdt, tag="et")
                        spl = max(1, min(EXP_SPLIT, w))
                        bnd = [round(w * 512 * k / spl) for k in range(spl + 1)]
                        for k in range(spl):
                            nc.scalar.activation(
                                et[:, bnd[k] : bnd[k + 1]],
                                ps[:, bnd[k] : bnd[k + 1]],
                                mybir.ActivationFunctionType.Exp,
                            )
                        if mode == "nopv":
                            continue
                        for i, ck in enumerate(g):
                            nc.tensor.matmul(
                                u_ps[0 : D + 1, :],
                                lhsT=xh_sb[
                                    :,
                                    (n * KC + ck) * (D + 1) : (n * KC + ck + 1) * (D + 1),
                                ],
                                rhs=et[:, 512 * i : 512 * (i + 1)],
                                start=(ck == 0),
                                stop=(ck == KC - 1),
                            )
                    if mode == "nopv":
                        continue
                    # drain this node's unnormalized U^T to SBUF (frees the
                    # PSUM bank) and ship it to DRAM; the trivial divide/
                    # transpose happens on host during gather
                    u_sb = etp.tile([128, 512], f32, tag="usb", bufs=2, name="u_sb")
                    nc.vector.tensor_copy(u_sb[0 : D + 1, :], u_ps[0 : D + 1, :])
                    nc.sync.dma_start(
                        uout[0 : D + 1, 512 * n : 512 * (n + 1)], u_sb[0 : D + 1, :]
                    )
                if rep_marker and mode != "nopv":
                    mark = mrkp.tile([1, 4], f32, tag="mark")
                    nc.vector.memset(mark[:], float(rep_i))
                    nc.sync.dma_start(uout[D + 1 : D + 2, 0:4], mark[:])
    _split_sync_waits(nc)
    return nc


def _host_prep(x, A, gc_weight, bn_gamma, bn_beta, bn_mean, bn_var):
    x = np.asarray(x, np.float32)
    A = np.asarray(A, np.float32)
    W = np.asarray(gc_weight, np.float32)
    scale = np.asarray(bn_gamma, np.float32) / np.sqrt(
        np.asarray(bn_var, np.float32) + BN_EPS
    )
    d_half = 0.5 * np.eye(N, dtype=np.float32)
    a0 = np.ones((N, N), np.float32) - np.eye(N, dtype=np.float32)
    adj = d_half @ (a0 + A) @ d_half
    wk = 0.5 * (adj[0] + adj[1])                      # [N]
    cn = (wk * scale).astype(np.float32)              # [N]
    offset = float(
        np.sum(wk * (np.asarray(bn_beta, np.float32)
                     - np.asarray(bn_mean, np.float32) * scale))
    )
    bias_vec = (offset * W.sum(axis=0)).astype(np.float32)  # [D]

    xt = x.transpose(1, 2, 0)                         # [N, D, B] (view)
    xh = np.empty((N, B, D + 1), np.float32)
    for n in range(N):
        xh[n, :, :D] = (x[:, n, :] @ W) * cn[n]
        xh[n, :, D] = 1.0
    xh = np.ascontiguousarray(xh.reshape(N, KC, 128, D + 1))
    return xt, xh, bias_vec


def _in_maps(xt, xh):
    maps = []
    for c in range(NCORES):
        xtc = np.ascontiguousarray(
            np.concatenate([xt, xt[:, :, c * R : (c + 1) * R]], axis=2)
        )  # [N, D, B + R]
        maps.append({"xtc": xtc, "xh": xh})
    return maps


def _finish(uouts, bias_vec):
    """Host gather: normalize U (divide by the folded rowsum), transpose to
    [rows, D], sum nodes, concatenate core slabs, add the BN/adjacency bias."""
    out = np.empty((B, D), np.float32)
    for c in range(NCORES):
        u = uouts[c]                                   # [D+2, 512*N]
        acc = np.zeros((512, D), np.float32)
        for n in range(N):
            un = u[: D + 1, 512 * n : 512 * (n + 1)]   # [11, 512]
            acc += (un[:D] / un[D]).T
        out[c * R : (c + 1) * R] = acc
    return out + bias_vec[None, :]


def kernel(**inputs) -> np.ndarray:
    assert inputs["x"].shape == (B, N, D)
    xt, xh, bias_vec = _host_prep(**inputs)
    nc = build_nc(rep=1)
    res = run_bass_kernel_spmd(nc, _in_maps(xt, xh), list(range(NCORES)))
    return _finish(
        [res.results[c]["uout"] for c in range(NCORES)], bias_vec
    ).astype(np.float32)



# revision 14
# speedup vs baseline: 7.6853x; 7.6853x over previous
"""Self-contained Trainium2 Bass kernel for nn_GCNResnet (batch-attention GCN).

Math (reference collapse):
  out[b,:] = sum_n softmax(X_n X_n^T)[b,:] @ Yh_n[:, :10] / (softmax-denom)
with Yh_n = [c_n*(X_n@W) | 1]; c_n and a constant offset fold BN(eval) +
adjacency + GCN + avgpool scalars. The softmax normalizer rides along as the
ones column of Yh: U_n = exp(S_n) @ Yh_n, out_n = U[:, :10] / U[:, 10].
(no max-subtraction: |scores| <= ~45, exp stays in fp32/bf16 range).

Sharding: row-slab parallel over 8 cores, 512 query rows per core, no
collectives. Per core per node: scores S^T chunks [128 keys x 512 rows] on the
PE (bf16 operands, 2-way row tiling on strips 0/64). The exp is the throughput
wall, so it is SPLIT between two engines working different chunk groups:
  - ACT: exp LUT, psum fp32 -> et bf16
  - DVE: Schraudolph bit-trick exp: bf16(exp(s)) ~= bitcast16(i16(s*A16+B16)),
    one tensor_scalar (mult+add, truncating int16 convert) per group
PV uses et as the *stationary* operand: U[128q, 11] += et[128k,128q]^T-op
@ xh[128k, 11] -- only 11 moving cycles per matmul, FWL bf16 weight loads.
U accumulates in one PSUM bank per node ([128, 44] = 4 q-subchunks x 11),
drained by DVE + DMA; the trivial divide/transpose happens on host.

PSUM: two scores buffers (4 banks + 3 banks) double-buffer PE vs ACT/DVE,
U accumulator 1 bank. Engine assignment alternates per group to balance
ACT (1.2 GHz) vs DVE (0.96 GHz) exp throughput.
"""

import os
import sys

if "/opt/trn_rl_repo" not in sys.path:
    sys.path.insert(0, "/opt/trn_rl_repo")

import numpy as np
import ml_dtypes

import concourse.bass as bass
import concourse.mybir as mybir
from concourse import tile
from concourse.bass_utils import run_bass_kernel_spmd
from concourse.vector_clock import ScopedClock

B, N, D = 4096, 3, 10
NCORES = 8
R = B // NCORES            # 512 query rows per core
KC = B // 128              # 32 key chunks of 128
QS = R // 128              # 4 query subchunks of 128
BN_EPS = 1e-5

# Schraudolph constants for bf16 bit patterns via int16 (truncating convert)
C16 = 9.0
A16 = float(2**7 / np.log(2))
B16 = float(127 * 128 - C16)

# schedule knobs
GROUP_PATTERN = (3, 2, 2)  # chunk-group widths, pool = group_idx % 3
ROW_STRIPS = (0, 32, 64)   # row-tile partition offsets (3-way)
ET_BUFS = 2                # whole-node et tiles in flight
# engine per group index within a node: pairs hide the PE-fill latency
# (an engine's next buffer is PE-filled while it drains the previous one);
# ACT(1.2GHz) and DVE(0.96GHz) get equal chunk counts on average.
ENGINE_PATTERN = ("act", "act", "dve", "dve")

# Local CoreSim (profiling) asserts every instruction carries tile-framework
# sem updates, which the walrus waitsplit workaround nops lack. The sim path
# has no walrus, so the workarounds are disabled there.
_SIM_MODE = bool(os.environ.get("KERNEL_SIM"))


def _patched_drain_and_barrier(self, tick_clock, wait_clock):
    # Walrus in this container rejects >1 sync-wait on a CTRL-class
    # instruction; absorb the tail-drain waits into SP nops, one wait each.
    nc = self.nc
    probe = nc.sync.nop()
    wait_clock.add_sem_waits(probe.ins, ScopedClock({None: tick_clock.global_clock}))
    si = probe.ins.sync_info
    waits = list(si.on_wait) if si is not None else []
    upds = list(si.on_update) if si is not None else []
    probe.ins.sync_info = mybir.SyncInfo(on_wait=waits[:1], on_update=upds)
    for w in waits[1:]:
        n = nc.sync.nop()
        n.ins.sync_info = mybir.SyncInfo(on_wait=[w], on_update=[])
    nc.sync.drain()
    nc.all_engine_barrier()
    assert self.sems is not None
    popped = nc._tile_sem_poison_stack.pop()
    assert popped is self._sem_poison
    nc.clear_and_free_semaphores(list(self.sems.allocated().values()))
    nc.all_engine_barrier()


if not _SIM_MODE:
    tile.TileContext._drain_and_barrier = _patched_drain_and_barrier

_MAX_WAITS = 1
_waitsplit_ctr = [0]


def _split_sync_waits(nc):
    """Walrus here allows very few sync-waits per instruction. Move excess
    waits onto same-engine no-ops placed immediately before the instruction
    (engine streams are in-order, so semantics are preserved)."""
    if _SIM_MODE:
        return
    for f in nc.m.functions:
        for bb in f.blocks:
            new = []
            changed = False
            for inst in bb.instructions:
                si = inst.sync_info
                waits = list(si.on_wait) if si is not None else []
                if len(waits) > _MAX_WAITS:
                    changed = True
                    for w in waits[:-_MAX_WAITS]:
                        _waitsplit_ctr[0] += 1
                        nop = mybir.InstNoOp(
                            name=f"I-waitsplit-{_waitsplit_ctr[0]}", ins=[], outs=[]
                        )
                        nop.engine = inst.engine
                        nop.sync_info = mybir.SyncInfo(on_wait=[w], on_update=[])
                        new.append(nop)
                    inst.sync_info = mybir.SyncInfo(
                        on_wait=waits[-_MAX_WAITS:], on_update=list(si.on_update)
                    )
                new.append(inst)
            if changed:
                bb.instructions = new


def _groups():
    """(chunk list, psum pool idx, engine) per group of one node."""
    gs, c, gi = [], 0, 0
    while c < KC:
        w = min(GROUP_PATTERN[gi % len(GROUP_PATTERN)], KC - c)
        eng = ENGINE_PATTERN[gi % len(ENGINE_PATTERN)]
        gs.append((list(range(c, c + w)), gi % len(GROUP_PATTERN), eng))
        c += w
        gi += 1
    return gs


def build_nc(rep: int = 1, rep_marker: bool = False, mode: str = "full") -> bass.Bass:
    """One-core SPMD program: full keys + this core's 512-query slab.

    mode: "full" (loads+compute per rep), "loads" (DMAs only per rep),
    "compute" (loads once, compute per rep), "nopv" (no PV/output).
    """
    f32 = mybir.dt.float32
    bf16 = mybir.dt.bfloat16
    i16 = mybir.dt.int16
    nc = bass.Bass()

    # xt: per node [10, B + R] bf16; cols 0..B-1 all keys, cols B.. this
    # core's query slab. xh: PV moving operand [128, 11] bf16 per key chunk.
    xt_d = nc.declare_dram_parameter("xt", [N, D, B + R], bf16, isOutput=False)
    xh_d = nc.declare_dram_parameter("xh", [N, KC, 128, D + 1], bf16, isOutput=False)
    UW = N * QS * (D + 1)
    uout = nc.declare_dram_parameter("uout", [128, UW + 4], f32, isOutput=True)

    groups = _groups()
    wmax = max(GROUP_PATTERN)

    with tile.TileContext(nc) as tc:
        with (
            tc.tile_pool(name="xtp", bufs=2) as xtp,
            tc.tile_pool(name="xhp", bufs=2) as xhp,
            tc.tile_pool(name="etp", bufs=ET_BUFS) as etp,
            tc.tile_pool(name="mrk", bufs=1) as mrkp,
            tc.tile_pool(name="pssA", bufs=1, space="PSUM") as pssA,
            tc.tile_pool(name="pssB", bufs=1, space="PSUM") as pssB,
            tc.tile_pool(name="pssC", bufs=1, space="PSUM") as pssC,
            tc.tile_pool(name="psu", bufs=1, space="PSUM") as psu,
        ):
            ps_pools = [pssA, pssB, pssC]
            xt_sb = xh_sb = None
            for rep_i in range(rep):
                if mode != "compute" or rep_i == 0:
                    xt_sb = [
                        xtp.tile([128, B + R], bf16, tag=f"xt{n}", name=f"xt{n}")
                        for n in range(N)
                    ]
                    xh_sb = xhp.tile([128, N * KC * (D + 1)], bf16, tag="xh")
                    xh_r = xh_sb[:].rearrange("p (n c d) -> p n c d", n=N, c=KC)
                    # Load DMAs spread over the 3 DMA-capable queues so they
                    # run in parallel on HW. Node 0's three strips go first
                    # (one per queue -> ~3.6us head); later nodes use SP/Pool
                    # only, keeping ACT (the exp wall) free after the head.
                    sched = [
                        (nc.sync, 0, 0), (nc.gpsimd, 0, 1), (nc.scalar, 0, 2),
                        (nc.sync, 0, None), (nc.gpsimd, 1, None),
                        (nc.sync, 1, 0), (nc.gpsimd, 1, 1),
                        (nc.sync, 1, 2), (nc.gpsimd, 2, 0),
                        (nc.sync, 2, 1), (nc.gpsimd, 2, 2),
                        (nc.sync, 2, None),
                    ]
                    for eng, n, si in sched:
                        if si is None:
                            eng.dma_start(
                                xh_r[:, n], xh_d[n].rearrange("c p d -> p c d")
                            )
                        else:
                            po = ROW_STRIPS[si]
                            eng.dma_start(xt_sb[n][po : po + D, :], xt_d[n])
                if mode == "loads":
                    continue

                for n in range(N):
                    # whole-node exp(S^T) tile: 32 chunks of [128 keys, 512 q]
                    et = etp.tile([128, 512 * KC], bf16, tag="et", name="et")
                    for g, pidx, eng in groups:
                        w = len(g)
                        pool = ps_pools[pidx]
                        gw = GROUP_PATTERN[pidx]
                        ps = pool.tile(
                            [128, 512 * gw], f32, tag=f"s{pidx}", name=f"s{pidx}"
                        )
                        for i, ck in enumerate(g):
                            po = ROW_STRIPS[i % len(ROW_STRIPS)]
                            nc.tensor.matmul(
                                ps[:, 512 * i : 512 * (i + 1)],
                                lhsT=xt_sb[n][po : po + D, 128 * ck : 128 * (ck + 1)],
                                rhs=xt_sb[n][po : po + D, B : B + R],
                                tile_position=(po, 0),
                            )
                        e_sl = et[:, 512 * g[0] : 512 * (g[0] + w)]
                        if eng == "act":
                            nc.scalar.activation(
                                e_sl,
                                ps[:, : 512 * w],
                                mybir.ActivationFunctionType.Exp,
                            )
                        else:
                            nc.vector.tensor_scalar(
                                e_sl.bitcast(i16),
                                ps[:, : 512 * w],
                                A16,
                                B16,
                                mybir.AluOpType.mult,
                                mybir.AluOpType.add,
                            )
                    if mode == "nopv":
                        continue
                    # PV: U[128q, 11] += et_chunk^T-op @ xh_chunk, one query
                    # subchunk sweep at a time (single PSUM bank; start=True
                    # zeroes the whole bank, so drain between sweeps)
                    u_sb = etp.tile([128, QS * (D + 1)], f32, tag="usb", bufs=2,
                                    name="u_sb")
                    for qs in range(QS):
                        u_ps = psu.tile([128, D + 1], f32, tag="u")
                        for ck in range(KC):
                            nc.tensor.matmul(
                                u_ps[:],
                                lhsT=et[:, 512 * ck + 128 * qs : 512 * ck + 128 * (qs + 1)],
                                rhs=xh_r[:, n, ck],
                                start=(ck == 0),
                                stop=(ck == KC - 1),
                            )
                        nc.vector.tensor_copy(
                            u_sb[:, (D + 1) * qs : (D + 1) * (qs + 1)], u_ps[:]
                        )
                    nc.sync.dma_start(
                        uout[:, QS * (D + 1) * n : QS * (D + 1) * (n + 1)], u_sb[:]
                    )
                if rep_marker and mode != "nopv":
                    mark = mrkp.tile([1, 4], f32, tag="mark")
                    nc.vector.memset(mark[:], float(rep_i))
                    nc.sync.dma_start(uout[0:1, UW : UW + 4], mark[:])
    _split_sync_waits(nc)
    return nc


def _host_prep(x, A, gc_weight, bn_gamma, bn_beta, bn_mean, bn_var):
    x = np.asarray(x, np.float32)
    A = np.asarray(A, np.float32)
    W = np.asarray(gc_weight, np.float32)
    scale = np.asarray(bn_gamma, np.float32) / np.sqrt(
        np.asarray(bn_var, np.float32) + BN_EPS
    )
    d_half = 0.5 * np.eye(N, dtype=np.float32)
    a0 = np.ones((N, N), np.float32) - np.eye(N, dtype=np.float32)
    adj = d_half @ (a0 + A) @ d_half
    wk = 0.5 * (adj[0] + adj[1])                      # [N]
    cn = (wk * scale).astype(np.float32)              # [N]
    offset = float(
        np.sum(wk * (np.asarray(bn_beta, np.float32)
                     - np.asarray(bn_mean, np.float32) * scale))
    )
    bias_vec = (offset * W.sum(axis=0)).astype(np.float32)  # [D]

    xt = np.ascontiguousarray(x.transpose(1, 2, 0))   # [N, D, B]
    xh = np.empty((N, B, D + 1), np.float32)
    for n in range(N):
        xh[n, :, :D] = (x[:, n, :] @ W) * cn[n]
        xh[n, :, D] = 1.0
    xh = np.ascontiguousarray(xh.reshape(N, KC, 128, D + 1)).astype(
        ml_dtypes.bfloat16
    )
    return xt, xh, bias_vec


def _in_maps(xt, xh):
    maps = []
    for c in range(NCORES):
        xtc = np.ascontiguousarray(
            np.concatenate([xt, xt[:, :, c * R : (c + 1) * R]], axis=2)
        ).astype(ml_dtypes.bfloat16)                   # [N, D, B + R]
        maps.append({"xt": xtc, "xh": xh})
    return maps


def _finish(uouts, bias_vec):
    """Host gather: normalize U (divide by the folded denominator column),
    sum nodes, concatenate core slabs, add the BN/adjacency bias."""
    out = np.empty((B, D), np.float32)
    for c in range(NCORES):
        u = np.asarray(uouts[c], np.float32)           # [128, N*QS*11 (+4)]
        acc = np.zeros((R, D), np.float32)
        for n in range(N):
            for qs in range(QS):
                blk = u[:, (D + 1) * (QS * n + qs) : (D + 1) * (QS * n + qs + 1)]
                acc[qs * 128 : (qs + 1) * 128] += blk[:, :D] / blk[:, D:]
        out[c * R : (c + 1) * R] = acc
    return out + bias_vec[None, :]


def kernel(**inputs) -> np.ndarray:
    assert inputs["x"].shape == (B, N, D)
    xt, xh, bias_vec = _host_prep(**inputs)
    nc = build_nc(rep=1)
    res = run_bass_kernel_spmd(nc, _in_maps(xt, xh), list(range(NCORES)))
    return _finish(
        [res.results[c]["uout"] for c in range(NCORES)], bias_vec
    ).astype(np.float32)
